# revision 19
# baseline (speedup 1.0000x reference)
"""Channel-attention (CAM) kernel for Trainium2, 8 NeuronCores.

Reference computation (per batch b):
    A   = x[b].reshape(L, C)            # L = 48^3 = 110592, C = 256
    G   = A^T A                          # [C, C] Gram matrix
    S   = softmax(G, axis=-1)
    out = gamma * (A @ S) + x[b]

Sharding: L-parallel across the 8 cores (each core owns L/8 rows of both
batches).  Each core computes a partial Gram over its shard; a per-batch
bf16 AllReduce completes the [C, C] Grams; every core redundantly
computes softmax (tiny), scales it by gamma, then computes its shard of
A @ (gamma*S) + A with a bf16 residual (max rel err ~4e-3, within the
2e-2 gate; gamma scaling of S makes the attention term exact for
gamma=0).

v2 vs the previous version: single HBM read.  x is loaded once (fp32,
1.57 MB supertiles of [128 part, 12 rows, 256 ch]), converted to bf16
and kept fully resident in SBUF (108 KB/partition for both batches);
phase 2 re-reads nothing.  Per-core HBM traffic drops from 70.8 MB to
the 56.6 MB floor (read x + write out once each).

Engine-queue discipline (FIFO queues make emission order matter):
  sync    = x loads, Gram staging to cc_in, out stores (loads and
            stores are time-disjoint in this schedule)
  gpsimd  = gamma broadcast, the two AllReduce triggers, and the
            post-AR gf fetches (they sit behind the AR wait anyway)
  vector  = fp32->bf16 converts, softmax max/recip/mul, residual adds
            (tensor_tensor reading y-PSUM directly, so MM2 needs no
            separate drain)
  scalar  = Gram AR staging copies, transpose-PSUM drains (batched
            [128,4,128] copies), softmax exp + s_bf scaling
The softmax dataflow is placed so an AllReduce wait can never block the
load/convert path: both ARs have ~15 us of slack.

PSUM budget (8 banks): gram[b] = [128,2,256] f32 = 1 bank x 2 batches;
tp (transpose staging) [128,4,128] = 1 bank x 2 bufs; y (MM2 out)
[128,4,256] = 2 banks x 2 bufs.
"""

import numpy as np
from contextlib import ExitStack

import concourse.bass as bass
import concourse.tile as tile
from concourse import bacc, mybir
from concourse.bass import ts
from concourse.bass_utils import run_bass_kernel_spmd
from concourse.masks import make_identity

F32 = mybir.dt.float32
BF16 = mybir.dt.bfloat16
AF = mybir.ActivationFunctionType

N_CORES = 8
B = 2
L = 48 * 48 * 48          # 110592
C = 256
L_SH = L // N_CORES       # 13824 rows per core per batch
ROWS = B * L_SH           # 27648 rows per core
P = 128
RPP = 12                  # rows per partition per supertile
SROWS = P * RPP           # 1536 rows per supertile
SPB = L_SH // SROWS       # 9 supertiles per batch
S_TOT = B * SPB           # 18 supertiles per core

_CACHE: dict = {}


def _build():
    nc = bacc.Bacc(
        "TRN2", target_bir_lowering=False, debug=False, num_devices=N_CORES
    )
    x_dram = nc.dram_tensor("x", [ROWS, C], F32, kind="ExternalInput")
    g_dram = nc.dram_tensor("gamma", [1, 1], F32, kind="ExternalInput")
    o_dram = nc.dram_tensor("out", [ROWS, C], F32, kind="ExternalOutput")
    cc_in = [
        nc.dram_tensor(f"cc_in{b}", [2 * P, C], BF16, kind="Internal")
        for b in range(B)
    ]
    cc_out = [
        nc.dram_tensor(
            f"cc_out{b}", [2 * P, C], BF16, kind="Internal",
            addr_space="Shared",
        )
        for b in range(B)
    ]
    cc_w_in = nc.dram_tensor("cc_w_in", [P, 16], BF16, kind="Internal")
    cc_w_out = nc.dram_tensor(
        "cc_w_out", [P, 16], BF16, kind="Internal", addr_space="Shared"
    )
    X, GAM, OUT = x_dram.ap(), g_dram.ap(), o_dram.ap()

    def x_super(s):
        return X[ts(s, SROWS), :].rearrange("(p j) c -> p j c", j=RPP)

    def o_super(s):
        return OUT[ts(s, SROWS), :].rearrange("(p j) c -> p j c", j=RPP)

    with tile.TileContext(nc) as tc, ExitStack() as octx:
        constp = octx.enter_context(tc.tile_pool(name="const", bufs=1))
        ident = constp.tile([P, P], BF16, name="ident", tag="ident")
        gam_sb = constp.tile([1, 1], F32, name="gam_sb", tag="gam_sb")
        gam_bc = constp.tile([P, 1], F32, name="gam_bc", tag="gam_bc")
        warm = constp.tile([P, 16], BF16, name="warm", tag="warm")
        s_bf = [
            constp.tile([P, C], BF16, name=f"sbf{i}", tag=f"sbf{i}")
            for i in range(4)
        ]
        gsb = [
            constp.tile([P, 2, C], BF16, name=f"gsb{b}", tag=f"gsb{b}")
            for b in range(B)
        ]

        xop = octx.enter_context(tc.tile_pool(name="xo", bufs=5))
        # xb ([128,12,256] bf16) and at ([128,24,128] bf16) are the same
        # 6 KB/partition slot; one pool+tag lets freed xb slots become at
        # slots (xb dies at its twork, at dies at its ywork, so total
        # occupancy never exceeds 18 + pipeline slack)
        ap_ = octx.enter_context(tc.tile_pool(name="ar", bufs=19))
        smp = octx.enter_context(tc.tile_pool(name="sm", bufs=2))

        def setup():
            # emitted after the first x loads so they hit the rings first.
            # The warmup AllReduce absorbs the cross-core launch stagger +
            # first-collective entry cost during the load phase, so the
            # real ARs start within ~2 us of their triggers (collectives
            # run back-to-back on the CC engine).
            nc.gpsimd.memset(warm[:], 0.0)
            nc.gpsimd.dma_start(cc_w_in.ap()[:, :], warm[:])
            nc.gpsimd.collective_compute(
                "AllReduce",
                mybir.AluOpType.add,
                replica_groups=[list(range(N_CORES))],
                ins=[cc_w_in.ap()[:, :]],
                outs=[cc_w_out.ap()[:, :]],
            )
            make_identity(nc, ident[:])
            nc.sync.dma_start(gam_sb[:], GAM[:, :])
            nc.gpsimd.partition_broadcast(gam_bc[:], gam_sb[:])
        psg = octx.enter_context(tc.tile_pool(name="psg", bufs=1, space="PSUM"))
        pst = octx.enter_context(tc.tile_pool(name="pst", bufs=2, space="PSUM"))
        psy = octx.enter_context(tc.tile_pool(name="psy", bufs=2, space="PSUM"))

        g_ps = [
            psg.tile([P, 2, C], F32, name=f"gps{b}", tag=f"gps{b}")
            for b in range(B)
        ]

        a_res: dict = {}
        at_res: dict = {}

        def p1(s):
            # SWDGE cast-load: fp32 HBM -> bf16 SBUF directly (the cast
            # happens in the DMA datapath), no staging or convert ops
            b = s // SPB
            xb = ap_.tile([P, RPP, C], BF16, name="xb", tag="xb")
            nc.gpsimd.dma_start(xb[:], x_super(s))
            a_res[s] = xb
            sfirst = s % SPB == 0
            slast = s % SPB == SPB - 1
            for j in range(RPP):
                first = sfirst and j == 0
                last = slast and j == RPP - 1
                nc.tensor.matmul(
                    g_ps[b][:, 0, :], xb[:, j, 0:P], xb[:, j, :],
                    start=first, stop=last,
                )
                nc.tensor.matmul(
                    g_ps[b][:, 1, :], xb[:, j, P:C], xb[:, j, :],
                    start=first, stop=last,
                )

        def stage_and_ar(b):
            nc.scalar.activation(gsb[b][:], g_ps[b][:], AF.Copy)
            for m in range(2):
                nc.sync.dma_start(cc_in[b].ap()[ts(m, P), :], gsb[b][:, m, :])
            nc.gpsimd.collective_compute(
                "AllReduce",
                mybir.AluOpType.add,
                replica_groups=[list(range(N_CORES))],
                ins=[cc_in[b].ap()[:, :]],
                outs=[cc_out[b].ap()[:, :]],
            )

        gf_t: dict = {}

        def softmax_gf(b):
            # gf fetches on gpsimd: they queue right behind the AR
            # trigger, so the AR wait blocks nothing else
            for m in range(2):
                gf = smp.tile([P, C], BF16, name="gf", tag=f"gf{m}")
                nc.gpsimd.dma_start(gf[:], cc_out[b].ap()[ts(m, P), :])
                gf_t[(b, m)] = gf

        def softmax_rest(b):
            # sequential softmax for batch 0: DVE+ACT queues are idle at
            # this point, so the zigzag chain resolves immediately
            for m in range(2):
                i = 2 * b + m
                gf = gf_t.pop((b, m))
                nmx = smp.tile([P, 1], F32, name="nmx", tag="nmx")
                nc.vector.tensor_reduce(
                    nmx[:], gf[:],
                    axis=mybir.AxisListType.X,
                    op=mybir.AluOpType.max,
                    negate=True,
                )
                ex = smp.tile([P, C], F32, name="ex", tag="ex")
                ssum = smp.tile([P, 1], F32, name="ssum", tag="ssum")
                nc.scalar.activation(
                    ex[:], gf[:], AF.Exp, bias=nmx[:], scale=1.0, accum_out=ssum[:]
                )
                inv = smp.tile([P, 1], F32, name="inv", tag="inv")
                nc.vector.reciprocal(inv[:], ssum[:])
                sc = smp.tile([P, 1], F32, name="sc", tag="sc")
                nc.vector.tensor_mul(sc[:], inv[:], gam_bc[:])
                nc.scalar.activation(s_bf[i][:], ex[:], AF.Copy, scale=sc[:])
                # fold the residual into the attention matrix:
                # M = gamma*S + I  =>  out = A @ M  (no separate add)
                nc.vector.tensor_add(
                    s_bf[i][:, ts(m, P)], s_bf[i][:, ts(m, P)], ident[:]
                )

        def twork(s):
            # A^T for supertile s: PE identity-matmuls into tp PSUM,
            # batched scalar-engine drains into a resident bf16 tile
            xb = a_res[s]
            at = ap_.tile([P, 2 * RPP, P], BF16, name="at", tag="xb")
            at_res[s] = at
            for q in range(RPP // 2):
                tp = pst.tile([P, 4, P], F32, name="tp", tag="tp")
                for u in range(2):
                    j = 2 * q + u
                    for h in range(2):
                        sl = 2 * u + h
                        nc.tensor.matmul(
                            tp[:, sl, :], xb[:, j, ts(h, P)], ident[:],
                            start=(sl == 0), stop=(sl == 3),
                        )
                if q % 2 == 0:
                    nc.scalar.activation(at[:, ts(q, 4), :], tp[:], AF.Copy)
                else:
                    nc.vector.tensor_copy(at[:, ts(q, 4), :], tp[:])

        def ywork(s):
            # out = A @ (gamma*S + I).  The y drain is a PSUM -> SBUF
            # bf16 copy (2x DVE mode, ~half the fp32 cost) split across
            # DVE and ACT; the store DMA upcasts bf16 -> fp32 in the
            # SWDGE datapath.  bf16(A)@I is exactly bf16-representable,
            # so the round-trip adds no error for the residual term.
            b = s // SPB
            a_res.pop(s)
            at = at_res.pop(s)
            ot = xop.tile([P, RPP, C], BF16, name="ot", tag="xo")
            for g in range(RPP // 4):
                y = psy.tile([P, 4, C], F32, name="y", tag="y")
                for jj in range(4):
                    j = 4 * g + jj
                    for h in range(2):
                        nc.tensor.matmul(
                            y[:, jj, :], at[:, 2 * j + h, :], s_bf[2 * b + h][:],
                            start=(jj % 2 == 0 and h == 0),
                            stop=(jj % 2 == 1 and h == 1),
                        )
                if (s + g) % 2 == 0:
                    nc.vector.tensor_copy(ot[:, ts(g, 4), :], y[:])
                else:
                    nc.scalar.activation(ot[:, ts(g, 4), :], y[:], AF.Copy)
            nc.gpsimd.dma_start(o_super(s), ot[:])

        # ---------------- schedule ----------------
        p1(0)
        p1(1)
        setup()
        for s in range(2, SPB):
            p1(s)
        stage_and_ar(0)
        for i in range(SPB):
            p1(SPB + i)
            if i in (1, 3, 5, 7):
                twork((i - 1) // 2)    # s = 0..3
        softmax_gf(0)
        stage_and_ar(1)
        # all remaining transposes run in the AllReduce dead zone: each
        # twork(s) releases xb[s], so pool occupancy stays flat at 18.
        # softmax_rest(0) is threaded between the twork groups so its AR0
        # wait blocks the evac queues only right when AR0 completes, and
        # MM2-s0 sits behind only the last two transposes in the PE FIFO.
        for k in range(4, 14):
            twork(k)
        softmax_rest(0)
        for k in range(14, 16):
            twork(k)
        for i in range(S_TOT):
            ywork(i)
            if i < 2:
                twork(16 + i)
            if i == 5:
                softmax_gf(1)      # late enough that its AR1 wait blocks
                                   # only odd stores that are not yet due
            if i == 6:
                softmax_rest(1)    # early-b0 evac is DVE, late-b0 is ACT,
                                   # so the DVE queue reaches this chain
                                   # right as AR1 completes

    nc.compile()
    return nc


def _get_nc():
    if "nc" not in _CACHE:
        _CACHE["nc"] = _build()
    return _CACHE["nc"]


def make_in_maps(inputs: dict) -> list:
    x3 = np.asarray(inputs["x"], dtype=np.float32).reshape(B, L, C)
    gam = np.asarray(inputs["gamma"], dtype=np.float32).reshape(1, 1)
    in_maps = []
    for k in range(N_CORES):
        shard = np.ascontiguousarray(
            x3[:, k * L_SH : (k + 1) * L_SH, :]
        ).reshape(ROWS, C)
        in_maps.append({"x": shard, "gamma": gam})
    return in_maps


def kernel(x: np.ndarray, gamma: np.ndarray, **_kw) -> np.ndarray:
    nc = _get_nc()
    x = np.asarray(x, dtype=np.float32)
    orig_shape = x.shape
    in_maps = make_in_maps({"x": x, "gamma": gamma})
    res = run_bass_kernel_spmd(nc, in_maps, core_ids=list(range(N_CORES)))
    out = np.empty((B, L, C), dtype=np.float32)
    for k in range(N_CORES):
        out[:, k * L_SH : (k + 1) * L_SH, :] = res.results[k]["out"].reshape(
            B, L_SH, C
        )
    return out.reshape(orig_shape)
